# revision 1
# baseline (speedup 1.0000x reference)
"""Bipartite matcher kernel for Trainium2 (8 NeuronCores).

Input:  x [512, 200000] fp32 IoU matrix (N=512 ground truths, M=200000 anchors).
Output: new_match [512] int32.

Strategy
--------
The O(N*M) device work is reduced to two segmented fp32 max-reduce passes per
column-shard (M sharded 8 ways):
  - rbm[n, b]  = max over 512-column block b of row n           (row side)
  - colg[g, m] = max over 32-row group g of column m            (col side)
The column side uses tensor_reduce(apply_transpose=True): the DVE 32x32
stream-transpose front-end turns the partition-axis (row) reduction into a
free-axis reduction directly from the natural row-major layout - no PE
transposes, no PSUM.

All argmax indices are recovered exactly on the host by scanning only the
winning 512-column block (rows) / 32-row group (columns), then the cheap
O(N+M) segment-max/scatter logic of the reference runs in numpy.
"""

import numpy as np

N = 512
M = 200000
NCORES = 8
M_SH = M // NCORES          # 25000 real columns per core
SUPER_W = 4096              # supertile width (columns)
N_FULL_ST = 6               # 6 * 4096 = 24576
LAST_W = 512                # + 512 -> 25088
M_PAD = N_FULL_ST * SUPER_W + LAST_W  # 25088
ROW_BLK = 512               # row-side column-block size
NBLK = M_PAD // ROW_BLK     # 49
COL_GRP = 32                # col-side row-group size
NCG = M_PAD // COL_GRP      # 784
PAD_VAL = -1.0
EPS = np.float32(1e-12)
FOLD_COLS = False  # GPSIMD/DMA folding rejected by this walrus build
TTR_ROWS = False   # tensor_tensor_reduce passes CoreSim but faults on HW

_CACHE: dict = {}


def _build_nc(m_pad=M_PAD, n_rows=N, loop_k=1, fold_cols=False, ttr_rows=False):
    """Build the per-core Bass program (SPMD, no collectives).

    loop_k > 1 wraps the whole body in an on-device For_i that re-processes
    the same data; used only for slope-based device-time measurement.
    fold_cols: GPSIMD pre-folds row-chunk pairs with pairwise max so the DVE
    column reduce touches half the elements (DVE is the bottleneck engine);
    the host then scans 64 candidate rows per column instead of 32."""
    from concourse import bacc, mybir
    from concourse.tile import TileContext

    f32 = mybir.dt.float32
    n_chunks = n_rows // 128
    nblk = m_pad // ROW_BLK
    ncg = m_pad // COL_GRP

    # Bacc (not plain Bass): its compile() runs generate_event_semaphores,
    # which splits multi-wait sync lists to satisfy the TRN2 one-wait-per-
    # instruction constraint that walrus enforces.
    nc = bacc.Bacc(None, target_bir_lowering=False)
    x_sh = nc.declare_dram_parameter("x_sh", [n_rows, m_pad], f32, isOutput=False)
    n_cg_chunks = n_chunks // 2 if fold_cols else n_chunks
    if loop_k > 1:
        # unused input whose shape encodes loop_k: makes each loop variant's
        # HLO structurally distinct so no compilation-cache layer can hand
        # one variant another's executable (slope-bench integrity)
        nc.declare_dram_parameter("k_tag", [1, loop_k], f32, isOutput=False)
    rbm = nc.declare_dram_parameter("rbm", [n_rows, nblk], f32, isOutput=True)
    colg = nc.declare_dram_parameter(
        "colg", [n_cg_chunks, 128, ncg], f32, isOutput=True
    )

    # supertile (base, width) list
    tiles = []
    base = 0
    while base < m_pad:
        w = min(SUPER_W, m_pad - base)
        tiles.append((base, w))
        base += w

    with TileContext(nc) as tc:
        with (
            tc.tile_pool(name="x", bufs=6) as xpool,
            tc.tile_pool(name="outs", bufs=1) as opool,
        ):
            rbm_t = [
                opool.tile([128, nblk], f32, name=f"rbm{c}", tag=f"rbm{c}")
                for c in range(n_chunks)
            ]
            colg_t = [
                opool.tile([128, ncg], f32, name=f"colg{c}", tag=f"colg{c}")
                for c in range(n_cg_chunks)
            ]
            scrap_t = [
                opool.tile([128, ROW_BLK // 2], f32, name=f"scr{c}", tag=f"scr{c}")
                for c in range(n_chunks if ttr_rows else 0)
            ]

            def colg_reduce(src_ap, cc, b0, w):
                # per-column maxes over 32-row groups via the DVE 32x32
                # stream-transpose front-end
                nc.vector.tensor_reduce(
                    out=colg_t[cc][:, b0 // COL_GRP:(b0 + w) // COL_GRP],
                    in_=src_ap.rearrange("p (k j) -> p k j", j=COL_GRP),
                    axis=mybir.AxisListType.X,
                    op=mybir.AluOpType.max,
                    apply_transpose=True,
                )

            def body():
                for (b0, w) in tiles:
                    ts = []
                    for c in range(n_chunks):
                        t = xpool.tile([128, w], f32, name="xt", tag="x")
                        nc.sync.dma_start(
                            out=t[:], in_=x_sh[c * 128:(c + 1) * 128, b0:b0 + w]
                        )
                        ts.append(t)
                        # row side: per-512-col block maxes
                        if ttr_rows:
                            # fused 2-port max: reads both block halves in one
                            # streaming pass (2 elems/cycle vs reduce's 1)
                            h = ROW_BLK // 2
                            for b in range(w // ROW_BLK):
                                o = b * ROW_BLK
                                nc.vector.tensor_tensor_reduce(
                                    out=scrap_t[c][:, :],
                                    in0=t[:, o:o + h],
                                    in1=t[:, o + h:o + ROW_BLK],
                                    scale=1.0,
                                    scalar=-2.0,
                                    op0=mybir.AluOpType.max,
                                    op1=mybir.AluOpType.max,
                                    accum_out=rbm_t[c][
                                        :, (b0 + o) // ROW_BLK:(b0 + o) // ROW_BLK + 1
                                    ],
                                )
                        else:
                            nc.vector.tensor_reduce(
                                out=rbm_t[c][:, b0 // ROW_BLK:(b0 + w) // ROW_BLK],
                                in_=t[:].rearrange("p (b j) -> p b j", j=ROW_BLK),
                                axis=mybir.AxisListType.X,
                                op=mybir.AluOpType.max,
                            )
                        if not fold_cols:
                            colg_reduce(t[:], c, b0, w)
                    if fold_cols:
                        # Fold chunk pairs with a DMA dst-reduce (CCE max in
                        # the DMA engines - no compute-engine cost): after
                        # t0's row reduce, t0 <- max(t0, t1) in place, then
                        # the column reduce reads the folded tile.
                        for f in range(n_chunks // 2):
                            t0, t1 = ts[2 * f], ts[2 * f + 1]
                            nc.gpsimd.dma_start(
                                out=t0[:], in_=t1[:], accum_op=mybir.AluOpType.max
                            )
                            colg_reduce(t0[:], f, b0, w)

            if loop_k == 1:
                body()
            else:
                with tc.For_i(0, loop_k, 1):
                    body()

            for c in range(n_chunks):
                nc.sync.dma_start(out=rbm[c * 128:(c + 1) * 128, :], in_=rbm_t[c][:])
            for cc in range(n_cg_chunks):
                nc.sync.dma_start(out=colg[cc, :, :], in_=colg_t[cc][:])
    nc.compile()
    return nc


def _get_nc():
    if "nc" not in _CACHE:
        _CACHE["nc"] = _build_nc(fold_cols=FOLD_COLS, ttr_rows=TTR_ROWS)
    return _CACHE["nc"]


def _device_outputs(x):
    """Run the Bass kernel on 8 cores; return (rbm_all, colg_all) per core."""
    from concourse.bass_utils import run_bass_kernel_spmd

    in_maps = []
    for c in range(NCORES):
        sh = np.full((N, M_PAD), PAD_VAL, np.float32)
        sh[:, :M_SH] = x[:, c * M_SH:(c + 1) * M_SH]
        in_maps.append({"x_sh": sh})
    bkr = run_bass_kernel_spmd(_get_nc(), in_maps, list(range(NCORES)))
    _CACHE["last_bkr"] = bkr  # exec_time_ns/profile for the test harness
    res = bkr.results
    ncg_chunks = 2 if FOLD_COLS else 4
    rbm_all = [np.asarray(res[c]["rbm"]).reshape(N, NBLK) for c in range(NCORES)]
    colg_all = [
        np.asarray(res[c]["colg"]).reshape(ncg_chunks, 128, NCG)
        for c in range(NCORES)
    ]
    return rbm_all, colg_all


def _combine(x, rbm_all, colg_all):
    """Exact reconstruction of the reference output from block/group maxes."""
    n, m = x.shape
    n_grp = n // COL_GRP  # 16 row-groups of 32

    # ---- column side: colmax + first-argmax per column --------------------
    fold = colg_all[0].shape[0] == 2
    n_cgc = colg_all[0].shape[0]
    n_g = n_cgc * 4
    # colg[cc, 32A+i, k] covers local col 32k+i; group g = 4*cc + A
    cm = np.concatenate(
        [
            colg_all[c]
            .reshape(n_cgc, 4, COL_GRP, NCG)
            .transpose(0, 1, 3, 2)
            .reshape(n_g, M_PAD)[:, :M_SH]
            for c in range(NCORES)
        ],
        axis=1,
    )  # [n_g, M]
    colmax = cm.max(axis=0)                        # [M] exact fp32 col max
    hits = cm == colmax[None, :]
    nhit = hits.sum(axis=0)
    first_g = hits.argmax(0)
    if not fold:
        # group g covers rows [32g, 32g+32): group order == row order, so the
        # first-hit group + first hit inside it is the exact argmax.
        rows_idx = first_g[None, :] * COL_GRP + np.arange(COL_GRP)[:, None]
        sub = x[rows_idx, np.arange(m)[None, :]]   # [32, M] gather
        ct = first_g * COL_GRP + (sub == colmax[None, :]).argmax(0)
    else:
        # group g = 4f+A covers rows [256f+32A,+32) u [256f+128+32A,+32)
        f_, A_ = np.divmod(first_g, 4)
        base = 256 * f_ + 32 * A_
        off = np.arange(COL_GRP)
        rows_idx = np.concatenate(
            [base[None, :] + off[:, None], base[None, :] + 128 + off[:, None]]
        )  # [64, M], ascending rows
        sub = x[rows_idx, np.arange(m)[None, :]]
        ct = rows_idx[
            (sub == colmax[None, :]).argmax(0), np.arange(m)
        ]
        # columns where several groups tie at colmax: group order is not row
        # order under folding, so recover the exact first row by full scan
        bad = np.where(nhit >= 2)[0]
        if bad.size:
            ct[bad] = np.asarray(x[:, bad]).argmax(axis=0)

    # ---- row side: rmax + first-argmax per row ----------------------------
    rbm_cat = np.concatenate(rbm_all, axis=1)      # [512, 8*49]
    rmax = rbm_cat.max(axis=1)
    first_b = (rbm_cat == rmax[:, None]).argmax(1)
    bp = np.empty(n, np.int64)                     # best_prior_idx / pargmax
    for i in range(n):
        core, blk = divmod(first_b[i], NBLK)
        c0 = blk * ROW_BLK
        w = min(ROW_BLK, M_SH - c0)
        seg = x[i, core * M_SH + c0: core * M_SH + c0 + w]
        bp[i] = core * M_SH + c0 + int((seg == rmax[i]).argmax())

    # ---- reference's segment/scatter logic (O(N+M), numpy) ----------------
    jr = np.arange(n, dtype=np.int64)
    forced = np.full(m, -1, np.int64)
    np.maximum.at(forced, bp, jr)
    match = np.where(forced >= 0, forced, ct)      # [M]

    forced2 = np.full(n, -1, np.int64)
    np.maximum.at(forced2, match, np.arange(m, dtype=np.int64))
    hit2 = np.bincount(match, minlength=n) > 0

    out = forced2.copy()
    need = np.where(~hit2)[0]
    for i in need:
        mask_i = np.count_nonzero((x[i] + EPS) >= colmax)
        out[i] = bp[i] if mask_i > 0 else -1
    return out.astype(np.int32)


def kernel(x):
    x = np.ascontiguousarray(np.asarray(x, dtype=np.float32))
    rbm_all, colg_all = _device_outputs(x)
    return _combine(x, rbm_all, colg_all)



# revision 2
# speedup vs baseline: 2.3694x; 2.3694x over previous
"""Bipartite matcher kernel for Trainium2 (8 NeuronCores).

Input:  x [512, 200000] fp32 IoU matrix (N=512 ground truths, M=200000 anchors).
Output: new_match [512] int32.

Strategy
--------
The device work is two max-reduction summaries per column-shard (M sharded 8
ways), computed over a HOST-QUANTIZED uint16 copy of the matrix (monotone
16-bit quantization => half the HBM traffic of fp32, and exact index recovery
on the host by rescanning only the small candidate sets in fp32):

  - rbm[r, B]   = q-max over 512-column block B of row r       (row side)
  - colg[g, cc] = q-max over (32-row band g x 8-column class)  (col side)

Most of the reduction work runs as tensor_tensor max folds, which the DVE
executes in 2x_1p mode (2 results/cycle for 16-bit dtypes) - twice the rate
of tensor_reduce (1/cycle, no perf modes). The fold tree is SHARED between
the row and column sides:

    t [128p, (4 chunks x 8 blocks) x 512] --m2--> x256 --m4--> x128 --m8--> x64
      row side:  m8 --m16--> x32 --tensor_reduce--> block maxes (512-col blocks)
      col side:  m8 --transpose_tensor_reduce--> 32-row-band maxes of 8-col
                 classes (columns {j8 + 64k} of one block fold together; the
                 host disambiguates by rescanning candidate bands in fp32)

Cycles per input element drop from 2.0 (two fp32 reduce passes) to ~0.66,
putting the kernel at the HBM roofline for the 16-bit stream.

Exactness: quantization is monotone, so any row block / (band, class) patch
containing the true fp32 max also achieves the quantized max. The host
gathers all candidate patches, rescans them in fp32, and reproduces the
reference's first-argmax semantics exactly. The final O(N+M) segment-max /
scatter logic runs in numpy as before.
"""

import numpy as np

N = 512
M = 200000
NCORES = 8
M_SH = M // NCORES          # 25000 real columns per core
SUPER_W = 4096              # supertile width (columns)
N_FULL_ST = 6               # 6 * 4096 = 24576
LAST_W = 512                # + 512 -> 25088
M_PAD = N_FULL_ST * SUPER_W + LAST_W  # 25088
ROW_BLK = 512               # row-side column-block size
NBLK = M_PAD // ROW_BLK     # 49
NCHUNK = 4                  # 512 rows / 128 partitions
CLS = 8                     # columns folded per class (m8 level)
BAND = 32                   # rows per column-side band
NBAND = N // BAND           # 16
EPS = np.float32(1e-12)

DT = "u16"                  # "u16" (quantized) | "f16" (fallback)
QSCALE = np.float32(65536.0)

RBM_COLS = NCHUNK * (M_PAD // ROW_BLK)       # 196 = 6*32 + 4
COLG_COLS = 2 * RBM_COLS                     # 392

_CACHE: dict = {}


def _supertiles():
    tiles = []
    base = 0
    while base < M_PAD:
        w = min(SUPER_W, M_PAD - base)
        tiles.append((base, w))
        base += w
    return tiles


def _np_dtype():
    return np.uint16 if DT == "u16" else np.float16


def _quant_np(v):
    """Monotone fp32 -> 16-bit map; must match the device input exactly."""
    v = np.asarray(v, np.float32)
    if DT == "u16":
        q = (v * QSCALE).astype(np.uint32)     # exact *2^16 then floor
        return np.minimum(q, 65535).astype(np.uint16)
    return v.astype(np.float16)


def _build_nc():
    """Per-core Bass program (SPMD, no collectives).

    Bacc (not plain Bass): its compile() runs generate_event_semaphores,
    which splits multi-wait sync lists to satisfy the TRN2 one-wait-per-
    instruction constraint that walrus enforces."""
    from concourse import bacc, mybir
    from concourse.tile import TileContext

    dt = mybir.dt.uint16 if DT == "u16" else mybir.dt.float16
    nc = bacc.Bacc(None, target_bir_lowering=False)
    x_sh = nc.declare_dram_parameter("x_sh", [128, NCHUNK, M_PAD], dt, isOutput=False)
    rbm = nc.declare_dram_parameter("rbm", [128, RBM_COLS], dt, isOutput=True)
    colg = nc.declare_dram_parameter("colg", [128, COLG_COLS], dt, isOutput=True)

    with TileContext(nc) as tc:
        with (
            tc.tile_pool(name="x", bufs=3) as xpool,
            tc.tile_pool(name="m2", bufs=2) as m2pool,
            tc.tile_pool(name="m4", bufs=2) as m4pool,
            tc.tile_pool(name="m8", bufs=2) as m8pool,
            tc.tile_pool(name="m16", bufs=2) as m16pool,
            tc.tile_pool(name="outs", bufs=1) as opool,
        ):
            rbm_t = opool.tile([128, RBM_COLS], dt, name="rbm_t", tag="rbm")
            colg_t = opool.tile([128, COLG_COLS], dt, name="colg_t", tag="colg")
            rb = cb = 0
            for (b0, w) in _supertiles():
                s = NCHUNK * w // ROW_BLK        # superblocks (chunk, block)
                t = xpool.tile([128, NCHUNK * w], dt, name="xt", tag="x")
                nc.sync.dma_start(
                    out=t[:].rearrange("p (c w) -> p c w", w=w),
                    in_=x_sh[:, :, b0:b0 + w],
                )
                v = t[:].rearrange("p (s j) -> p s j", j=ROW_BLK)
                m2 = m2pool.tile([128, s * 256], dt, name="m2", tag="m2")
                m2v = m2[:].rearrange("p (s j) -> p s j", j=256)
                nc.vector.tensor_max(m2v, v[:, :, 0:256], v[:, :, 256:512])
                m4 = m4pool.tile([128, s * 128], dt, name="m4", tag="m4")
                m4v = m4[:].rearrange("p (s j) -> p s j", j=128)
                nc.vector.tensor_max(m4v, m2v[:, :, 0:128], m2v[:, :, 128:256])
                m8 = m8pool.tile([128, s * 64], dt, name="m8", tag="m8")
                m8v = m8[:].rearrange("p (s j) -> p s j", j=64)
                nc.vector.tensor_max(m8v, m4v[:, :, 0:64], m4v[:, :, 64:128])
                # col side: 32-row-band maxes of 8-col classes via the DVE
                # 32x32 stream-transpose front-end
                nc.vector.tensor_reduce(
                    out=colg_t[:, cb:cb + 2 * s],
                    in_=m8[:].rearrange("p (k j) -> p k j", j=32),
                    axis=mybir.AxisListType.X,
                    op=mybir.AluOpType.max,
                    apply_transpose=True,
                )
                # row side tail: one more 2x fold, then a plain reduce
                m16 = m16pool.tile([128, s * 32], dt, name="m16", tag="m16")
                m16v = m16[:].rearrange("p (s j) -> p s j", j=32)
                nc.vector.tensor_max(m16v, m8v[:, :, 0:32], m8v[:, :, 32:64])
                nc.vector.tensor_reduce(
                    out=rbm_t[:, rb:rb + s],
                    in_=m16v,
                    axis=mybir.AxisListType.X,
                    op=mybir.AluOpType.max,
                )
                rb += s
                cb += 2 * s
            nc.sync.dma_start(out=rbm[:, :], in_=rbm_t[:])
            nc.sync.dma_start(out=colg[:, :], in_=colg_t[:])
    nc.compile()
    return nc


def _get_nc():
    if "nc" not in _CACHE:
        _CACHE["nc"] = _build_nc()
    return _CACHE["nc"]


def _make_shard(xq, c):
    """Device input for core c: [128 partitions, 4 chunks, M_PAD cols]."""
    sh = np.zeros((128, NCHUNK, M_PAD), _np_dtype())
    sh[:, :, :M_SH] = (
        xq[:, c * M_SH:(c + 1) * M_SH].reshape(NCHUNK, 128, M_SH).transpose(1, 0, 2)
    )
    return np.ascontiguousarray(sh)


def _device_outputs(x):
    from concourse.bass_utils import run_bass_kernel_spmd

    xq = _quant_np(x)
    in_maps = [{"x_sh": _make_shard(xq, c)} for c in range(NCORES)]
    bkr = run_bass_kernel_spmd(_get_nc(), in_maps, list(range(NCORES)))
    _CACHE["last_bkr"] = bkr  # exec_time_ns/profile for the test harness
    res = bkr.results
    rbm_all = [np.asarray(res[c]["rbm"]).reshape(128, RBM_COLS) for c in range(NCORES)]
    colg_all = [
        np.asarray(res[c]["colg"]).reshape(128, COLG_COLS) for c in range(NCORES)
    ]
    return rbm_all, colg_all


def _colg_index_maps():
    """Per local column mloc: colg column index (per chunk) and partition row.

    colg layout written by the device, per supertile st (s superblocks):
      col = st_base + (ch * blocks_per_chunk + b) * 2 + h,  partition = 32A + i
    where b = block-in-chunk, j = col offset in block, j8 = j % 64,
    h = j8 // 32, i = j8 % 32, A = 32-row band within the chunk."""
    mloc = np.arange(M_SH)
    st = np.minimum(mloc // SUPER_W, N_FULL_ST)
    off = mloc - st * SUPER_W
    b = off // ROW_BLK
    j = off % ROW_BLK
    j8 = j % 64
    h = j8 // 32
    i_ = j8 % 32
    full = st < N_FULL_ST
    base = np.where(full, st * 64 + b * 2 + h, N_FULL_ST * 64 + h)
    chstep = np.where(full, 16, 2)   # (ch*8)*2 for full supertiles, ch*2 last
    return base, chstep, i_


def _combine(x, rbm_all, colg_all):
    """Exact reconstruction of the reference output from quantized maxes."""
    n, m = x.shape

    # ---- row side: exact first-argmax per row ----------------------------
    # decode rbm [128, 196] -> [512 rows, 49 blocks] per core
    rbm_rows = np.empty((N, NCORES * NBLK), _np_dtype())
    for core in range(NCORES):
        rt = rbm_all[core]
        arr = rt[:, :NCHUNK * 8 * N_FULL_ST].reshape(128, N_FULL_ST, NCHUNK, 8)
        for ch in range(NCHUNK):
            rows = slice(ch * 128, (ch + 1) * 128)
            cols = slice(core * NBLK, core * NBLK + 48)
            rbm_rows[rows, cols] = arr[:, :, ch, :].reshape(128, 48)
            rbm_rows[rows, core * NBLK + 48] = rt[:, NCHUNK * 8 * N_FULL_ST + ch]

    bp = np.empty(N, np.int64)
    rmax_q = rbm_rows.max(axis=1)
    for r in range(N):
        best_v = -np.inf
        best_idx = -1
        for cb_ in np.flatnonzero(rbm_rows[r] == rmax_q[r]):
            core, B = divmod(int(cb_), NBLK)
            c0 = B * ROW_BLK
            wreal = min(ROW_BLK, M_SH - c0)
            seg = x[r, core * M_SH + c0: core * M_SH + c0 + wreal]
            mv = seg.max()
            if mv > best_v:
                best_v = mv
                best_idx = core * M_SH + c0 + int((seg == mv).argmax())
        bp[r] = best_idx

    # ---- col side: exact colmax + first-argmax row per column ------------
    base, chstep, i_ = _colg_index_maps()
    bv = np.empty((NBAND, m), _np_dtype())       # band beta = ch*4 + A
    for core in range(NCORES):
        cg = colg_all[core]
        sl = slice(core * M_SH, (core + 1) * M_SH)
        for ch in range(NCHUNK):
            cols = base + ch * chstep
            for A in range(4):
                bv[ch * 4 + A, sl] = cg[32 * A + i_, cols]

    colsM = np.arange(m)
    band0 = bv.argmax(0)                          # first band at quantized max
    rows_idx = band0[None, :] * BAND + np.arange(BAND)[:, None]
    sub = x[rows_idx, colsM[None, :]]             # [32, M] exact values
    best_val = sub.max(0)
    best_row = band0 * BAND + (sub == best_val[None, :]).argmax(0)
    q1 = _quant_np(best_val)
    cand = bv >= q1[None, :]
    cand[band0, colsM] = False
    for beta in range(NBAND):
        cols_b = np.flatnonzero(cand[beta])
        if cols_b.size == 0:
            continue
        subb = x[beta * BAND:(beta + 1) * BAND, cols_b]
        mb = subb.max(0)
        rb_ = beta * BAND + (subb == mb[None, :]).argmax(0)
        cur_v = best_val[cols_b]
        cur_r = best_row[cols_b]
        upd = (mb > cur_v) | ((mb == cur_v) & (rb_ < cur_r))
        ii = cols_b[upd]
        best_val[ii] = mb[upd]
        best_row[ii] = rb_[upd]
    ct = best_row                                  # best_truth_idx per anchor
    colmax = best_val                              # exact fp32 col max

    # ---- reference's segment/scatter logic (O(N+M), numpy) ----------------
    jr = np.arange(n, dtype=np.int64)
    forced = np.full(m, -1, np.int64)
    np.maximum.at(forced, bp, jr)
    match = np.where(forced >= 0, forced, ct)      # [M]

    forced2 = np.full(n, -1, np.int64)
    np.maximum.at(forced2, match, np.arange(m, dtype=np.int64))
    hit2 = np.bincount(match, minlength=n) > 0

    out = forced2.copy()
    for i in np.where(~hit2)[0]:
        mask_i = np.count_nonzero((x[i] + EPS) >= colmax)
        out[i] = bp[i] if mask_i > 0 else -1
    return out.astype(np.int32)


def kernel(x):
    x = np.ascontiguousarray(np.asarray(x, dtype=np.float32))
    rbm_all, colg_all = _device_outputs(x)
    return _combine(x, rbm_all, colg_all)
